# revision 35
# baseline (speedup 1.0000x reference)
"""CodeSage attention (B=2, S=2048, H=1024, 16 heads x 64) on 8 Trainium2 cores.

Sharding: tensor-parallel over heads — 2 heads per core. Each core computes
its head-group's QKV projection, attention, and the c_proj partial product;
the host sums the 8 partials and adds c_proj_b + bv @ c_proj_w (the V-bias
contribution reduces to a constant output row, applied host-side exactly).

Device-side design (bf16 matmuls, fp32 accumulation):

phase 1 (k-outer for stationary reuse, two 4-block passes so the projection
accumulators only occupy 4 PSUM banks and the score pool can coexist):
    qT,kT [128=2*64, T] = Wslice^T @ hsT   (1/sqrt(hd) folded into wq host-side)
    V natural per key tile; the PSUM->aug copy fuses the exp(mask) row scale:
    aug[g] = [ v'_h0(64) | em64(64) | v'_h1(64) ]  where em = exp(mask),
    v' = v*em — this folds the additive mask into V/ones stationaries, making
    the exp bias-free and the sumexp mask-aware.

phase 2, per (batch, 512-query-block) i — software-pipelined and interleaved
so ScalarE (the exp pacer) never starves:
    A: scoresT[sk,sq] = kT-slices^T @ qT  (2-head row-tiled pairs, K=64)
       probs = exp(scoresT)               (ScalarE)
    B: aug matmuls (K=128): psA = [ctx_h0 | se_h0*64], psB = [se_h1*64 | ctx_h1]
    N: DVE copy + DMA realign of sumexp rows, approx-reciprocal, ctxn = ctx*rec
    C: c_proj partial: out_tile = ctxn^T @ wp -> DVE copy -> DMA out
    Emission: Q, K, A(0), V; then for i: ILV[B(i) ~ A(i+1)], N(i), C(i-1).
"""

import numpy as np
import ml_dtypes

B, S, H = 2, 2048, 1024
NH, HD = 16, 64
NCORES = 8
HPC = NH // NCORES          # heads per core = 2
DC = HPC * HD               # per-core head dims = 128
T = B * S                   # 4096 tokens
KC = H // 128               # 8 contraction chunks
NBLK = T // 512             # 8 column blocks of 512 tokens
SQB = S // 512              # 4 query blocks per batch
SKT = S // 128              # 16 key tiles per batch
NIT = B * SQB               # 8 pipelined phase-2 iterations

_CACHE = {}


def _build_nc():
    import concourse.mybir as mybir
    import concourse.tile as tile
    from concourse import bacc

    f32 = mybir.dt.float32
    bf16 = mybir.dt.bfloat16

    nc = bacc.Bacc("TRN2", target_bir_lowering=False, debug=False,
                   num_devices=NCORES)

    hsT_d = nc.dram_tensor("hsT", [H, T], bf16, kind="ExternalInput")
    wq_d = nc.dram_tensor("wq", [128, KC, DC], bf16, kind="ExternalInput")
    wk_d = nc.dram_tensor("wk", [128, KC, DC], bf16, kind="ExternalInput")
    wv_d = nc.dram_tensor("wv", [128, KC, DC], bf16, kind="ExternalInput")
    wp_d = nc.dram_tensor("wp", [DC, H], bf16, kind="ExternalInput")
    bq_d = nc.dram_tensor("bq", [DC, 1], f32, kind="ExternalInput")
    bk_d = nc.dram_tensor("bk", [DC, 1], f32, kind="ExternalInput")
    mask_d = nc.dram_tensor("mask", [B, S], f32, kind="ExternalInput")
    out_d = nc.dram_tensor("out", [T, H], f32, kind="ExternalOutput")

    EXP = mybir.ActivationFunctionType.Exp
    MULT = mybir.AluOpType.mult

    with tile.TileContext(nc) as tc:
        with (
            tc.tile_pool(name="const", bufs=1) as cpool,
            tc.tile_pool(name="qkv", bufs=1) as qpool,
            tc.tile_pool(name="probs", bufs=27) as ppool,
            tc.tile_pool(name="ctxn", bufs=2) as npool,
            tc.tile_pool(name="rcin", bufs=2) as ripool,
            tc.tile_pool(name="rec", bufs=2) as rpool,
            tc.tile_pool(name="ob", bufs=3) as opool,
            tc.tile_pool(name="ps_sc", bufs=3, space="PSUM") as pssc,
        ):
            wq_sb = cpool.tile([128, KC, DC], bf16)
            wk_sb = cpool.tile([128, KC, DC], bf16)
            wv_sb = cpool.tile([128, KC, DC], bf16)
            wp_sb = cpool.tile([DC, H], bf16)
            bq_sb = cpool.tile([DC, 1], f32)
            bk_sb = cpool.tile([DC, 1], f32)
            ones64 = cpool.tile([128, 64], bf16)
            mask_sb = cpool.tile([128, B, SKT], f32)
            em_sb = cpool.tile([128, B, SKT], f32)

            nc.sync.dma_start(wq_sb[:], wq_d.ap())
            nc.sync.dma_start(wk_sb[:], wk_d.ap())
            nc.sync.dma_start(wv_sb[:], wv_d.ap())
            nc.sync.dma_start(wp_sb[:], wp_d.ap())
            nc.sync.dma_start(bq_sb[:], bq_d.ap())
            nc.sync.dma_start(bk_sb[:], bk_d.ap())
            nc.sync.dma_start(mask_sb[:], mask_d.ap().rearrange("b (t p) -> p b t", p=128))
            nc.vector.memset(ones64[:], 1.0)
            nc.scalar.activation(em_sb[:], mask_sb[:], EXP)

            qT_sb = qpool.tile([128, T], bf16)   # rows 0:64 head0, 64:128 head1
            kT_sb = qpool.tile([128, T], bf16)
            # aug stationaries: per key tile g, [v'_h0 | em64 | v'_h1]
            aug_sb = qpool.tile([128, B * SKT, 192], bf16)

            # ---- phase-2 emission helpers ---------------------------------
            def emit_sc(i, skt):
                b = i // SQB
                sq0 = b * S + (i % SQB) * 512
                sk = slice(b * S + skt * 128, b * S + skt * 128 + 128)
                sq = slice(sq0, sq0 + 512)
                sc_ps = pssc.tile([128, 1024], f32, tag="sc", name="sc_ps")
                nc.tensor.matmul(sc_ps[:, 0:512], lhsT=kT_sb[0:64, sk],
                                 rhs=qT_sb[0:64, sq], start=True, stop=True,
                                 skip_group_check=True)
                nc.tensor.matmul(sc_ps[:, 512:1024], lhsT=kT_sb[64:128, sk],
                                 rhs=qT_sb[64:128, sq], start=True, stop=True,
                                 skip_group_check=True)
                pr = ppool.tile([128, 1024], bf16, tag="pr", name="pr")
                nc.scalar.activation(pr[:], sc_ps[:], EXP)
                return pr

            # ---- phase 1: QKV projection ----------------------------------
            with (
                tc.tile_pool(name="hs", bufs=1) as hpool,
                tc.tile_pool(name="ps1", bufs=1, space="PSUM") as ps1,
            ):
                hs_all = hpool.tile([128, KC, T], bf16)
                # quarter-chunk transfers, quarter-major: the first proj pass
                # (kT/qT blocks 0-1) only reads column-quarter 0 of every
                # chunk, so emitting all chunks' quarter-0 first lets scores
                # start after ~2 MB of input instead of all 8 MB.
                for q4 in range(4):
                    cs = slice(q4 * 1024, (q4 + 1) * 1024)
                    for k in range(KC):
                        nc.sync.dma_start(hs_all[:, k, cs],
                                          hsT_d.ap()[k * 128:(k + 1) * 128, cs])

                def proj_joint(lanes):
                    """k-major accumulation over several (w, dst, bias, blk)
                    lanes at once — paces with arriving hs chunks."""
                    ps = [ps1.tile([128, 512], f32, tag=f"b{j}", name=f"ps_j{j}")
                          for j in range(len(lanes))]
                    for k in range(KC):
                        for j, (w_sb, _, _, blk) in enumerate(lanes):
                            cols = slice(blk * 512, (blk + 1) * 512)
                            nc.tensor.matmul(ps[j][:], lhsT=w_sb[:, k, :],
                                             rhs=hs_all[:, k, cols],
                                             start=(k == 0), stop=(k == KC - 1),
                                             skip_group_check=True)
                    for j, (_, dst_sb, bias_sb, blk) in enumerate(lanes):
                        cols = slice(blk * 512, (blk + 1) * 512)
                        nc.vector.tensor_scalar_add(dst_sb[:, cols], ps[j][:],
                                                    bias_sb[:, 0:1])

                KL = (wk_sb, kT_sb, bk_sb)
                QL = (wq_sb, qT_sb, bq_sb)
                # 2-lane passes (ps1 = 2 banks, leaving 6 for the score pool);
                # batch-0 K and Q first so iteration-0 exp starts early.
                proj_joint([KL + (0,), KL + (1,)])
                proj_joint([QL + (0,), KL + (2,)])
                probs0 = [emit_sc(0, skt) for skt in range(SKT // 2)]
                proj_joint([KL + (3,), QL + (1,)])
                probs0 += [emit_sc(0, skt) for skt in range(SKT // 2, SKT)]
                proj_joint([QL + (2,), QL + (3,)])
                proj_joint([KL + (4,), KL + (5,)])
                proj_joint([QL + (4,), KL + (6,)])
                proj_joint([KL + (7,), QL + (5,)])
                proj_joint([QL + (6,), QL + (7,)])

                # V natural per key tile g; PSUM->aug copy fuses em scaling.
                # Bridge pairs of iteration-1 scores/exp into the V loop so
                # ScalarE doesn't starve between A(0) and the first ILV.
                bridge = []
                for g in range(B * SKT):
                    b, skt = g // SKT, g % SKT
                    em = em_sb[:, b, skt:skt + 1]
                    gc = slice(g * 128, (g + 1) * 128)
                    v_ps = ps1.tile([128, DC], f32, tag=f"b{g % 2}", name="v_ps")
                    for k in range(KC):
                        nc.tensor.matmul(v_ps[:], lhsT=hs_all[:, k, gc],
                                         rhs=wv_sb[:, k, :],
                                         start=(k == 0), stop=(k == KC - 1),
                                         skip_group_check=True)
                    # both 64-wide v halves in one op: out cols {0:64,128:192}
                    aug_v = aug_sb[:, g, :].rearrange("p (a b) -> p a b", a=3)[:, 0:3:2, :]
                    src_v = v_ps[:].rearrange("p (two c) -> p two c", two=2)
                    nc.vector.tensor_scalar_mul(aug_v, src_v, em)
                    nc.vector.tensor_scalar_mul(aug_sb[:, g, 64:128], ones64[:], em)
                    if NIT > 1 and g % 3 == 2 and len(bridge) < 10:
                        bridge.append(emit_sc(1, len(bridge)))
                        bridge.append(emit_sc(1, len(bridge)))

            # ---- phase 2: attention + c_proj ------------------------------
            with tc.tile_pool(name="ps_ab", bufs=1, space="PSUM") as psab:
                prev = None  # (ctxn tile, sq0) pending c_proj

                def emit_cproj(ctxn, sq0):
                    for t4 in range(4):
                        tok = slice(t4 * 128, (t4 + 1) * 128)
                        rows = slice(sq0 + t4 * 128, sq0 + (t4 + 1) * 128)
                        op = pssc.tile([128, 1024], f32, tag="sc", name="op")
                        nc.tensor.matmul(op[:, 0:512], lhsT=ctxn[:, tok],
                                         rhs=wp_sb[:, 0:512], start=True, stop=True,
                                         skip_group_check=True)
                        nc.tensor.matmul(op[:, 512:1024], lhsT=ctxn[:, tok],
                                         rhs=wp_sb[:, 512:1024], start=True, stop=True,
                                         skip_group_check=True)
                        ob = opool.tile([128, 1024], f32, tag="ob", name="ob")
                        nc.vector.tensor_copy(ob[:, 0:512], op[:, 0:512])
                        nc.vector.tensor_copy(ob[:, 512:1024], op[:, 512:1024])
                        nc.sync.dma_start(out_d.ap()[rows, :], ob[:])

                probs = probs0
                nxt = bridge
                for i in range(NIT):
                    b = i // SQB
                    sq0 = b * S + (i % SQB) * 512
                    # --- ILV: aug matmuls of i interleaved with scores/exp
                    # of i+1 (keeps ScalarE fed while PE runs aug) --------
                    psA = psab.tile([128, 512], f32, tag="pa", name="psA")
                    psB = psab.tile([128, 512], f32, tag="pb", name="psB")
                    for skt in range(SKT):
                        if i + 1 < NIT and skt % 4 == 0 and len(nxt) < SKT:
                            for _ in range(4):
                                nxt.append(emit_sc(i + 1, len(nxt)))
                        g = b * SKT + skt
                        st, sp = (skt == 0), (skt == SKT - 1)
                        nc.tensor.matmul(psA[:], lhsT=aug_sb[:, g, 0:128],
                                         rhs=probs[skt][:, 0:512], start=st, stop=sp,
                                         skip_group_check=True)
                        nc.tensor.matmul(psB[:], lhsT=aug_sb[:, g, 64:192],
                                         rhs=probs[skt][:, 512:1024], start=st, stop=sp,
                                         skip_group_check=True)
                        if skt == 6 and prev is not None:
                            emit_cproj(*prev)
                            prev = None
                    # --- N: normalize --------------------------------------
                    se_st = ripool.tile([128, 512], f32, tag="st", name="se_st")
                    nc.vector.tensor_copy(se_st[64:128, :], psA[64:128, :])
                    nc.vector.tensor_copy(se_st[0:64, :], psB[0:64, :])
                    rec_in = ripool.tile([128, 512], f32, tag="ri", name="rec_in")
                    nc.sync.dma_start(rec_in[0:64, :], se_st[64:128, :])
                    nc.sync.dma_start(rec_in[64:128, :], se_st[0:64, :])
                    rec = rpool.tile([128, 512], f32, tag="rc", name="rec")
                    nc.vector.reciprocal_approx_fast(rec[:], rec_in[:])
                    ctxn = npool.tile([128, 512], bf16, tag="cn", name="ctxn")
                    nc.vector.tensor_tensor(ctxn[0:64, :], psA[0:64, :],
                                            rec[0:64, :], op=MULT)
                    nc.vector.tensor_tensor(ctxn[64:128, :], psB[64:128, :],
                                            rec[64:128, :], op=MULT)
                    # --- C(i-1), if not already emitted mid-ILV ------------
                    if prev is not None:
                        emit_cproj(*prev)
                    prev = (ctxn, sq0)
                    probs = nxt
                    nxt = []

                emit_cproj(*prev)

    nc.compile()
    return nc


def _enable_ldw_opt():
    """Flip walrus's --enable-ldw-opt for our kernel's compile (LDWEIGHTS
    batching/acceleration); verified against the reference in test.py."""
    from concourse import bass_utils as _bu
    if getattr(_bu, "_ldw_opt_patched", False):
        return
    _orig = _bu.get_walrus_args

    def _patched(*a, **k):
        return [str(x).replace("--enable-ldw-opt=false", "--enable-ldw-opt=true")
                for x in _orig(*a, **k)]

    _bu.get_walrus_args = _patched
    _bu._ldw_opt_patched = True


def _get_nc():
    if "nc" not in _CACHE:
        _enable_ldw_opt()
        _CACHE["nc"] = _build_nc()
    return _CACHE["nc"]


def kernel(hidden_states, attention_mask, c_attn_w, c_attn_b, c_proj_w, c_proj_b):
    from concourse.bass_utils import run_bass_kernel_spmd

    bf16 = ml_dtypes.bfloat16
    hs = np.asarray(hidden_states, dtype=np.float32).reshape(T, H)
    hsT = np.ascontiguousarray(hs.T).astype(bf16)
    mask = np.ascontiguousarray(
        np.broadcast_to(
            np.asarray(attention_mask, dtype=np.float32).reshape(B, 1, 1, S)[:, 0, 0, :],
            (B, S),
        )
    )
    w = np.asarray(c_attn_w, dtype=np.float32)
    bqkv = np.asarray(c_attn_b, dtype=np.float32)
    wp_full = np.asarray(c_proj_w, dtype=np.float32)
    scale = 1.0 / np.sqrt(HD)

    def pack(a):  # [H, DC] -> [128, KC, DC], contiguous per-partition lines
        return np.ascontiguousarray(
            a.reshape(KC, 128, DC).transpose(1, 0, 2)).astype(bf16)

    in_maps = []
    for c in range(NCORES):
        lo, hi = c * DC, (c + 1) * DC
        in_maps.append({
            "hsT": hsT,
            "wq": pack(w[:, lo:hi] * scale),
            "wk": pack(w[:, H + lo:H + hi]),
            "wv": pack(w[:, 2 * H + lo:2 * H + hi]),
            "wp": np.ascontiguousarray(wp_full[lo:hi, :]).astype(bf16),
            "bq": np.ascontiguousarray((bqkv[lo:hi] * scale).reshape(DC, 1)),
            "bk": np.ascontiguousarray(bqkv[H + lo:H + hi].reshape(DC, 1)),
            "mask": mask,
        })

    res = run_bass_kernel_spmd(_get_nc(), in_maps, core_ids=list(range(NCORES)))
    _CACHE["last_result"] = res
    acc = np.zeros((T, H), dtype=np.float32)
    for c in range(NCORES):
        acc += res.results[c]["out"]
    # v-bias contributes the constant row bv @ c_proj_w (exact, host-side)
    bv_full = bqkv[2 * H:3 * H]
    acc += (bv_full @ wp_full + np.asarray(c_proj_b, dtype=np.float32))[None, :]
    return acc.reshape(B, S, H)


# revision 37
# speedup vs baseline: 1.1776x; 1.1776x over previous
"""CodeSage attention (B=2, S=2048, H=1024, 16 heads x 64) on 8 Trainium2 cores.

Sharding: tensor-parallel over heads — 2 heads per core. Each core computes
its head-group's QKV projection, attention, and the c_proj partial product;
the host sums the 8 partials and adds c_proj_b + bv @ c_proj_w (the V-bias
contribution reduces to a constant output row, applied host-side exactly).

Device-side design (bf16 matmuls, fp32 accumulation):

phase 1 (k-outer for stationary reuse, two 4-block passes so the projection
accumulators only occupy 4 PSUM banks and the score pool can coexist):
    qT,kT [128=2*64, T] = Wslice^T @ hsT   (1/sqrt(hd) folded into wq host-side)
    V natural per key tile; the PSUM->aug copy fuses the exp(mask) row scale:
    aug[g] = [ v'_h0(64) | em64(64) | v'_h1(64) ]  where em = exp(mask),
    v' = v*em — this folds the additive mask into V/ones stationaries, making
    the exp bias-free and the sumexp mask-aware.

phase 2, per (batch, 512-query-block) i — software-pipelined and interleaved
so ScalarE (the exp pacer) never starves:
    A: scoresT[sk,sq] = kT-slices^T @ qT  (2-head row-tiled pairs, K=64)
       probs = exp(scoresT)               (ScalarE)
    B: aug matmuls (K=128): psA = [ctx_h0 | se_h0*64], psB = [se_h1*64 | ctx_h1]
    N: DVE copy + DMA realign of sumexp rows, approx-reciprocal, ctxn = ctx*rec
    C: c_proj partial: out_tile = ctxn^T @ wp -> DVE copy -> DMA out
    Emission: Q, K, A(0), V; then for i: ILV[B(i) ~ A(i+1)], N(i), C(i-1).
"""

import numpy as np
import ml_dtypes

B, S, H = 2, 2048, 1024
NH, HD = 16, 64
NCORES = 8
HPC = NH // NCORES          # heads per core = 2
DC = HPC * HD               # per-core head dims = 128
T = B * S                   # 4096 tokens
KC = H // 128               # 8 contraction chunks
NBLK = T // 512             # 8 column blocks of 512 tokens
SQB = S // 512              # 4 query blocks per batch
SKT = S // 128              # 16 key tiles per batch
NIT = B * SQB               # 8 pipelined phase-2 iterations

_CACHE = {}


def _build_nc():
    import concourse.mybir as mybir
    import concourse.tile as tile
    from concourse import bacc

    f32 = mybir.dt.float32
    bf16 = mybir.dt.bfloat16

    nc = bacc.Bacc("TRN2", target_bir_lowering=False, debug=False,
                   num_devices=NCORES)

    hsT_d = nc.dram_tensor("hsT", [H, T], bf16, kind="ExternalInput")
    wq_d = nc.dram_tensor("wq", [128, KC, DC], bf16, kind="ExternalInput")
    wk_d = nc.dram_tensor("wk", [128, KC, DC], bf16, kind="ExternalInput")
    wv_d = nc.dram_tensor("wv", [128, KC, DC], bf16, kind="ExternalInput")
    wp_d = nc.dram_tensor("wp", [DC, H], bf16, kind="ExternalInput")
    bq_d = nc.dram_tensor("bq", [DC, 1], f32, kind="ExternalInput")
    bk_d = nc.dram_tensor("bk", [DC, 1], f32, kind="ExternalInput")
    mask_d = nc.dram_tensor("mask", [B, S], f32, kind="ExternalInput")
    out_d = nc.dram_tensor("out", [T, H], f32, kind="ExternalOutput")

    EXP = mybir.ActivationFunctionType.Exp
    MULT = mybir.AluOpType.mult

    with tile.TileContext(nc) as tc:
        with (
            tc.tile_pool(name="const", bufs=1) as cpool,
            tc.tile_pool(name="qkv", bufs=1) as qpool,
            tc.tile_pool(name="probs", bufs=27) as ppool,
            tc.tile_pool(name="ctxn", bufs=2) as npool,
            tc.tile_pool(name="rcin", bufs=2) as ripool,
            tc.tile_pool(name="rec", bufs=2) as rpool,
            tc.tile_pool(name="ob", bufs=3) as opool,
            tc.tile_pool(name="ps_sc", bufs=2, space="PSUM") as pssc,
        ):
            wq_sb = cpool.tile([128, KC, DC], bf16)
            wk_sb = cpool.tile([128, KC, DC], bf16)
            wv_sb = cpool.tile([128, KC, DC], bf16)
            wp_sb = cpool.tile([DC, H], bf16)
            bq_sb = cpool.tile([DC, 1], f32)
            bk_sb = cpool.tile([DC, 1], f32)
            ones64 = cpool.tile([128, 64], bf16)
            mask_sb = cpool.tile([128, B, SKT], f32)
            em_sb = cpool.tile([128, B, SKT], f32)

            nc.sync.dma_start(wq_sb[:], wq_d.ap())
            nc.sync.dma_start(wk_sb[:], wk_d.ap())
            nc.sync.dma_start(wv_sb[:], wv_d.ap())
            nc.sync.dma_start(bq_sb[:], bq_d.ap())
            nc.sync.dma_start(bk_sb[:], bk_d.ap())
            nc.vector.memset(ones64[:], 1.0)

            qT_sb = qpool.tile([128, T], bf16)   # rows 0:64 head0, 64:128 head1
            kT_sb = qpool.tile([128, T], bf16)
            # aug stationaries: per key tile g, [v'_h0 | em64 | v'_h1]
            aug_sb = qpool.tile([128, B * SKT, 192], bf16)

            # ---- phase-2 emission helpers ---------------------------------
            def emit_sc(i, skt):
                b = i // SQB
                sq0 = b * S + (i % SQB) * 512
                sk = slice(b * S + skt * 128, b * S + skt * 128 + 128)
                sq = slice(sq0, sq0 + 512)
                sc_ps = pssc.tile([128, 1024], f32, tag="sc", name="sc_ps")
                nc.tensor.matmul(sc_ps[:, 0:512], lhsT=kT_sb[0:64, sk],
                                 rhs=qT_sb[0:64, sq], start=True, stop=True,
                                 skip_group_check=True)
                nc.tensor.matmul(sc_ps[:, 512:1024], lhsT=kT_sb[64:128, sk],
                                 rhs=qT_sb[64:128, sq], start=True, stop=True,
                                 skip_group_check=True)
                pr = ppool.tile([128, 1024], bf16, tag="pr", name="pr")
                nc.scalar.activation(pr[:], sc_ps[:], EXP)
                return pr

            # ---- phase 1: QKV projection ----------------------------------
            with (
                tc.tile_pool(name="hs", bufs=1) as hpool,
                tc.tile_pool(name="ps1", bufs=1, space="PSUM") as ps1,
            ):
                hs_all = hpool.tile([128, KC, T], bf16)
                # quarter-chunk transfers, quarter-major: the first proj pass
                # (kT/qT blocks 0-1) only reads column-quarter 0 of every
                # chunk, so emitting all chunks' quarter-0 first lets scores
                # start after ~2 MB of input instead of all 8 MB.
                for q4 in range(4):
                    cs = slice(q4 * 1024, (q4 + 1) * 1024)
                    for k in range(KC):
                        nc.sync.dma_start(hs_all[:, k, cs],
                                          hsT_d.ap()[k * 128:(k + 1) * 128, cs])
                # deferred: not needed until the V loop / first c_proj
                nc.sync.dma_start(wp_sb[:], wp_d.ap())
                nc.sync.dma_start(mask_sb[:], mask_d.ap().rearrange("b (t p) -> p b t", p=128))
                nc.scalar.activation(em_sb[:], mask_sb[:], EXP)

                def proj_joint(lanes):
                    """k-major accumulation over several (w, dst, bias, blk)
                    lanes at once — paces with arriving hs chunks."""
                    ps = [ps1.tile([128, 512], f32, tag=f"b{j}", name=f"ps_j{j}")
                          for j in range(len(lanes))]
                    for k in range(KC):
                        for j, (w_sb, _, _, blk) in enumerate(lanes):
                            cols = slice(blk * 512, (blk + 1) * 512)
                            nc.tensor.matmul(ps[j][:], lhsT=w_sb[:, k, :],
                                             rhs=hs_all[:, k, cols],
                                             start=(k == 0), stop=(k == KC - 1),
                                             skip_group_check=True)
                    for j, (_, dst_sb, bias_sb, blk) in enumerate(lanes):
                        cols = slice(blk * 512, (blk + 1) * 512)
                        nc.vector.tensor_scalar_add(dst_sb[:, cols], ps[j][:],
                                                    bias_sb[:, 0:1])

                KL = (wk_sb, kT_sb, bk_sb)
                QL = (wq_sb, qT_sb, bq_sb)
                # batch-0 K and Q first; exp for iteration 0 starts as soon as
                # the needed kT/qT halves exist, while PE continues projecting.
                proj_joint([KL + (0,), KL + (1,), QL + (0,)])
                probs0 = [emit_sc(0, skt) for skt in range(SKT // 2)]
                proj_joint([KL + (2,), KL + (3,), QL + (1,)])
                probs0 += [emit_sc(0, skt) for skt in range(SKT // 2, SKT)]
                proj_joint([QL + (2,), QL + (3,)])
                proj_joint([KL + (4,), KL + (5,), QL + (4,), QL + (5,)])
                proj_joint([KL + (6,), KL + (7,), QL + (6,), QL + (7,)])

                # V natural per key tile g; PSUM->aug copy fuses em scaling.
                # Bridge pairs of iteration-1 scores/exp into the V loop so
                # ScalarE doesn't starve between A(0) and the first ILV.
                bridge = []
                for g in range(B * SKT):
                    b, skt = g // SKT, g % SKT
                    em = em_sb[:, b, skt:skt + 1]
                    gc = slice(g * 128, (g + 1) * 128)
                    v_ps = ps1.tile([128, DC], f32, tag=f"b{g % 4}", name="v_ps")
                    for k in range(KC):
                        nc.tensor.matmul(v_ps[:], lhsT=hs_all[:, k, gc],
                                         rhs=wv_sb[:, k, :],
                                         start=(k == 0), stop=(k == KC - 1),
                                         skip_group_check=True)
                    # both 64-wide v halves in one op: out cols {0:64,128:192}
                    aug_v = aug_sb[:, g, :].rearrange("p (a b) -> p a b", a=3)[:, 0:3:2, :]
                    src_v = v_ps[:].rearrange("p (two c) -> p two c", two=2)
                    nc.vector.tensor_scalar_mul(aug_v, src_v, em)
                    nc.vector.tensor_scalar_mul(aug_sb[:, g, 64:128], ones64[:], em)
                    if NIT > 1 and g % 3 == 2 and len(bridge) < 10:
                        bridge.append(emit_sc(1, len(bridge)))
                        bridge.append(emit_sc(1, len(bridge)))

            # ---- phase 2: attention + c_proj ------------------------------
            with tc.tile_pool(name="ps_ab", bufs=2, space="PSUM") as psab:
                prev = None  # (ctxn tile, sq0) pending c_proj

                def emit_cproj(ctxn, sq0):
                    for t4 in range(4):
                        tok = slice(t4 * 128, (t4 + 1) * 128)
                        rows = slice(sq0 + t4 * 128, sq0 + (t4 + 1) * 128)
                        op_a = psab.tile([128, 512], f32, tag="pa", name="op_a")
                        op_b = psab.tile([128, 512], f32, tag="pb", name="op_b")
                        nc.tensor.matmul(op_a[:], lhsT=ctxn[:, tok],
                                         rhs=wp_sb[:, 0:512], start=True, stop=True,
                                         skip_group_check=True)
                        nc.tensor.matmul(op_b[:], lhsT=ctxn[:, tok],
                                         rhs=wp_sb[:, 512:1024], start=True, stop=True,
                                         skip_group_check=True)
                        ob = opool.tile([128, 1024], f32, tag="ob", name="ob")
                        nc.vector.tensor_copy(ob[:, 0:512], op_a[:])
                        nc.vector.tensor_copy(ob[:, 512:1024], op_b[:])
                        nc.sync.dma_start(out_d.ap()[rows, :], ob[:])

                probs = probs0
                nxt = bridge
                for i in range(NIT):
                    b = i // SQB
                    sq0 = b * S + (i % SQB) * 512
                    # --- ILV: aug matmuls of i interleaved with scores/exp
                    # of i+1 (keeps ScalarE fed while PE runs aug) --------
                    psA = psab.tile([128, 512], f32, tag="pa", name="psA")
                    psB = psab.tile([128, 512], f32, tag="pb", name="psB")
                    for skt in range(SKT):
                        if i + 1 < NIT and skt % 2 == 0 and len(nxt) < SKT:
                            nxt.append(emit_sc(i + 1, len(nxt)))
                            nxt.append(emit_sc(i + 1, len(nxt)))
                        g = b * SKT + skt
                        st, sp = (skt == 0), (skt == SKT - 1)
                        nc.tensor.matmul(psA[:], lhsT=aug_sb[:, g, 0:128],
                                         rhs=probs[skt][:, 0:512], start=st, stop=sp,
                                         skip_group_check=True)
                        nc.tensor.matmul(psB[:], lhsT=aug_sb[:, g, 64:192],
                                         rhs=probs[skt][:, 512:1024], start=st, stop=sp,
                                         skip_group_check=True)
                        if skt == 4 and prev is not None:
                            emit_cproj(*prev)
                            prev = None
                    # --- N: normalize --------------------------------------
                    se_st = ripool.tile([128, 512], f32, tag="st", name="se_st")
                    nc.vector.tensor_copy(se_st[64:128, :], psA[64:128, :])
                    nc.vector.tensor_copy(se_st[0:64, :], psB[0:64, :])
                    rec_in = ripool.tile([128, 512], f32, tag="ri", name="rec_in")
                    nc.sync.dma_start(rec_in[0:64, :], se_st[64:128, :])
                    nc.sync.dma_start(rec_in[64:128, :], se_st[0:64, :])
                    rec = rpool.tile([128, 512], f32, tag="rc", name="rec")
                    nc.vector.reciprocal_approx_fast(rec[:], rec_in[:])
                    ctxn = npool.tile([128, 512], bf16, tag="cn", name="ctxn")
                    nc.vector.tensor_tensor(ctxn[0:64, :], psA[0:64, :],
                                            rec[0:64, :], op=MULT)
                    nc.vector.tensor_tensor(ctxn[64:128, :], psB[64:128, :],
                                            rec[64:128, :], op=MULT)
                    # --- C(i-1), if not already emitted mid-ILV ------------
                    if prev is not None:
                        emit_cproj(*prev)
                    prev = (ctxn, sq0)
                    probs = nxt
                    nxt = []

                emit_cproj(*prev)

    nc.compile()
    return nc


def _enable_ldw_opt():
    """Flip walrus's --enable-ldw-opt for our kernel's compile (LDWEIGHTS
    batching/acceleration); verified against the reference in test.py."""
    from concourse import bass_utils as _bu
    if getattr(_bu, "_ldw_opt_patched", False):
        return
    _orig = _bu.get_walrus_args

    def _patched(*a, **k):
        return [str(x).replace("--enable-ldw-opt=false", "--enable-ldw-opt=true")
                for x in _orig(*a, **k)]

    _bu.get_walrus_args = _patched
    _bu._ldw_opt_patched = True


def _get_nc():
    if "nc" not in _CACHE:
        _enable_ldw_opt()
        _CACHE["nc"] = _build_nc()
    return _CACHE["nc"]


def kernel(hidden_states, attention_mask, c_attn_w, c_attn_b, c_proj_w, c_proj_b):
    from concourse.bass_utils import run_bass_kernel_spmd

    bf16 = ml_dtypes.bfloat16
    hs = np.asarray(hidden_states, dtype=np.float32).reshape(T, H)
    hsT = np.ascontiguousarray(hs.T).astype(bf16)
    mask = np.ascontiguousarray(
        np.broadcast_to(
            np.asarray(attention_mask, dtype=np.float32).reshape(B, 1, 1, S)[:, 0, 0, :],
            (B, S),
        )
    )
    w = np.asarray(c_attn_w, dtype=np.float32)
    bqkv = np.asarray(c_attn_b, dtype=np.float32)
    wp_full = np.asarray(c_proj_w, dtype=np.float32)
    scale = 1.0 / np.sqrt(HD)

    def pack(a):  # [H, DC] -> [128, KC, DC], contiguous per-partition lines
        return np.ascontiguousarray(
            a.reshape(KC, 128, DC).transpose(1, 0, 2)).astype(bf16)

    in_maps = []
    for c in range(NCORES):
        lo, hi = c * DC, (c + 1) * DC
        in_maps.append({
            "hsT": hsT,
            "wq": pack(w[:, lo:hi] * scale),
            "wk": pack(w[:, H + lo:H + hi]),
            "wv": pack(w[:, 2 * H + lo:2 * H + hi]),
            "wp": np.ascontiguousarray(wp_full[lo:hi, :]).astype(bf16),
            "bq": np.ascontiguousarray((bqkv[lo:hi] * scale).reshape(DC, 1)),
            "bk": np.ascontiguousarray(bqkv[H + lo:H + hi].reshape(DC, 1)),
            "mask": mask,
        })

    res = run_bass_kernel_spmd(_get_nc(), in_maps, core_ids=list(range(NCORES)))
    _CACHE["last_result"] = res
    acc = np.zeros((T, H), dtype=np.float32)
    for c in range(NCORES):
        acc += res.results[c]["out"]
    # v-bias contributes the constant row bv @ c_proj_w (exact, host-side)
    bv_full = bqkv[2 * H:3 * H]
    acc += (bv_full @ wp_full + np.asarray(c_proj_b, dtype=np.float32))[None, :]
    return acc.reshape(B, S, H)
